# revision 4
# baseline (speedup 1.0000x reference)
"""Trainium2 Bass kernel for CrossDepthAttentionResidual.

Reference computation (L=12, B=2, S=2048, D=1024, DK=256):
    normalized = LayerNorm_D(states)                    # (L,B,S,D)
    query  = normalized[-1] @ Wq.T                      # (B,S,DK)
    keys   = normalized @ Wk.T                          # (L,B,S,DK)
    logits = einsum('bsk,lbsk->lbs', query, keys)/16    # (L,B,S)
    w      = softmax_l(logits)
    mixed  = einsum('lbs,lbsd->bsd', w, states)
    out    = g*states[-1] + (1-g)*mixed,  g = sigmoid(latest_gate)

Key algebraic rewrite: logits[l,n] = (Wq@norm11[n]) . (Wk@norm[l,n])
                                   = u[n] . norm[l,n]
with u[n] = Wk.T @ (Wq @ norm11[n]) computed once per position from the
*last* layer only.  The LN affine of layer l then folds into scalars:
    logits[l,n] = (r[l,n]*A[l,n] - r[l,n]*mu[l,n]*C1[n] + C2[n]) / 16
where A[l,n] = uw[n] . x[l,n]  (uw = u*ln_w), C1 = sum(uw), C2 = u . ln_b,
mu/r are the LN mean and rsqrt(var+eps).  This removes the big per-layer
keys matmul entirely: per-layer work is one pass of sum/sum-sq stats and
one fused dot product.  The final mix  out[n,:] = sum_l w'[l,n]*x[l,n,:]
(with gate folded into w'[11]) runs on the TensorEngine as
diag(w'_l).T @ x_l accumulated in PSUM.

Sharding: positions (b*S+s) are split contiguously across the 8 cores;
all compute is pointwise in position, so no collectives are needed.
"""

import math
from contextlib import ExitStack

import numpy as np

import concourse.bacc as bacc
import concourse.mybir as mybir
import concourse.tile as tile
from concourse import masks
from concourse.bass_utils import run_bass_kernel_spmd

L, B, S, D, DK = 12, 2, 2048, 1024, 256
N_CORES = 8
NTOT = B * S            # 4096 positions
NPC = NTOT // N_CORES   # 512 positions per core
P = 128                 # SBUF partitions
LN_EPS = 1e-5
SCALE = 1.0 / math.sqrt(DK)

F32 = mybir.dt.float32
BF16 = mybir.dt.bfloat16
U32 = mybir.dt.uint32
ALU = mybir.AluOpType
ACTF = mybir.ActivationFunctionType

RSQRT_MAGIC = 0x5F3759DF


def _rsqrt_newton(nc, pool, vpe, r_out, ncols, n_iter=3):
    """r_out = rsqrt(vpe) via bit-trick seed + Newton iterations (pure DVE).

    Avoids the ScalarEngine Sqrt table set (2.7us table switch + 65536-ULP
    budget).  vpe, r_out: [128, ncols] f32 SBUF tiles (contiguous).
    """
    magic = pool.tile([P, ncols], U32, tag="rs_magic")
    nc.vector.memset(magic[:], RSQRT_MAGIC)
    shifted = pool.tile([P, ncols], U32, tag="rs_shift")
    nc.vector.tensor_scalar(
        out=shifted[:], in0=vpe[:].bitcast(U32), scalar1=1, scalar2=None,
        op0=ALU.logical_shift_right,
    )
    yu = pool.tile([P, ncols], U32, tag="rs_seed")
    nc.vector.tensor_tensor(out=yu[:], in0=magic[:], in1=shifted[:], op=ALU.subtract)
    y = yu[:].bitcast(F32)
    t = pool.tile([P, ncols], F32, tag="rs_tmp")
    for _ in range(n_iter):
        # y <- y * (1.5 - 0.5 * vpe * y^2)
        nc.vector.tensor_tensor(out=t[:], in0=y, in1=y, op=ALU.mult)
        nc.vector.tensor_tensor(out=t[:], in0=t[:], in1=vpe[:], op=ALU.mult)
        nc.vector.tensor_scalar(
            out=t[:], in0=t[:], scalar1=-0.5, scalar2=1.5, op0=ALU.mult, op1=ALU.add,
        )
        nc.vector.tensor_tensor(out=t[:], in0=y, in1=t[:], op=ALU.mult)
        nc.vector.tensor_copy(r_out[:], t[:])
    return r_out


def build_program(npc, gate, use_affine):
    """Build the per-core SPMD Bass program.

    npc: positions handled by this core (multiple of 128).
    gate: float python scalar sigmoid(latest_gate), baked as immediates.
    use_affine: apply general ln_weight/ln_bias path (False when w==1,b==0).
    """
    assert npc % P == 0
    nt = npc // P
    g = float(gate)

    nc = bacc.Bacc("TRN2", target_bir_lowering=False, debug=False)

    x_dram = nc.dram_tensor("states_shard", [L, npc, D], F32, kind="ExternalInput")
    # wqt: [128, 8*256]; chunk c cols [c*256,(c+1)*256) holds Wq.T[c*128:(c+1)*128, :]
    wqt_dram = nc.dram_tensor("wqt", [P, 8 * DK], F32, kind="ExternalInput")
    # wk: [128, 2*1024]; chunk h cols [h*1024,...) holds Wk[h*128:(h+1)*128, :]
    wk_dram = nc.dram_tensor("wk", [P, 2 * D], F32, kind="ExternalInput")
    if use_affine:
        lnw_dram = nc.dram_tensor("lnw", [1, D], F32, kind="ExternalInput")
        lnb_dram = nc.dram_tensor("lnb", [1, D], F32, kind="ExternalInput")
    out_dram = nc.dram_tensor("out", [npc, D], F32, kind="ExternalOutput")

    with tile.TileContext(nc) as tc, ExitStack() as ctx:
        cpool = ctx.enter_context(tc.tile_pool(name="consts", bufs=1))
        xpool = ctx.enter_context(tc.tile_pool(name="x", bufs=2))
        bpool = ctx.enter_context(tc.tile_pool(name="xb16", bufs=4))
        b11pool = ctx.enter_context(tc.tile_pool(name="xb16_l11", bufs=2))
        scpool = ctx.enter_context(tc.tile_pool(name="ttr_scratch", bufs=4))
        spool = ctx.enter_context(tc.tile_pool(name="stats", bufs=2))
        mpool = ctx.enter_context(tc.tile_pool(name="mid", bufs=2))
        opool = ctx.enter_context(tc.tile_pool(name="outs", bufs=2))
        pT = ctx.enter_context(tc.tile_pool(name="psum_T", bufs=2, space="PSUM"))
        pQ = ctx.enter_context(tc.tile_pool(name="psum_q", bufs=2, space="PSUM"))
        pU = ctx.enter_context(tc.tile_pool(name="psum_u", bufs=1, space="PSUM"))
        pM = ctx.enter_context(tc.tile_pool(name="psum_m", bufs=1, space="PSUM"))

        # ---- constants ----
        ident = cpool.tile([P, P], F32)
        masks.make_identity(nc, ident[:])
        wqt = cpool.tile([P, 8 * DK], F32)
        nc.sync.dma_start(wqt[:], wqt_dram[:])
        wk = cpool.tile([P, 2 * D], F32)
        nc.sync.dma_start(wk[:], wk_dram[:])
        if use_affine:
            # broadcast ln params to all partitions (tiny, one-time)
            lnw_bc = cpool.tile([P, D], F32)
            nc.sync.dma_start(lnw_bc[0:1, :], lnw_dram[:])
            nc.gpsimd.partition_broadcast(lnw_bc[:], lnw_bc[0:1, :])
            lnb_bc = cpool.tile([P, D], F32)
            nc.sync.dma_start(lnb_bc[0:1, :], lnb_dram[:])
            nc.gpsimd.partition_broadcast(lnb_bc[:], lnb_bc[0:1, :])
            lnb_bc16 = cpool.tile([P, D], BF16)
            nc.vector.tensor_copy(lnb_bc16[:], lnb_bc[:])

        for t in range(nt):
            r0 = t * P
            # ---- load this position-tile for all layers (last layer first) ----
            x = xpool.tile([P, L, D], F32)
            for l in [L - 1] + list(range(L - 1)):
                nc.sync.dma_start(x[:, l, :], x_dram[l, r0:r0 + P, :])

            # per-layer LN stats ([count, mean, count*var] x even/odd halves)
            st = spool.tile([P, L, 12], F32, tag="st")
            ag = spool.tile([P, L, 2], F32, tag="ag")    # [mean, var]
            acol = spool.tile([P, L], F32, tag="acol")   # A = uw . x

            # ---------------- last-layer chain (critical path) ----------------
            xb11 = b11pool.tile([P, D], BF16)
            nc.scalar.copy(xb11[:], x[:, L - 1, :])
            nc.vector.bn_stats(st[:, L - 1, 0:6], xb11[:, 0:512])
            nc.vector.bn_stats(st[:, L - 1, 6:12], xb11[:, 512:1024])
            nc.vector.bn_aggr(ag[:, L - 1, :], st[:, L - 1, :])
            mu11 = ag[:, L - 1, 0:1]
            vpe11 = spool.tile([P, 1], F32, tag="vpe11")
            nc.vector.tensor_scalar(out=vpe11[:], in0=ag[:, L - 1, 1:2],
                                    scalar1=LN_EPS, scalar2=None, op0=ALU.add)
            r11 = spool.tile([P, 1], F32, tag="r11")
            _rsqrt_newton(nc, spool, vpe11, r11, 1)
            negmur = spool.tile([P, 1], F32, tag="negmur")
            nc.vector.tensor_tensor(out=negmur[:], in0=mu11, in1=r11[:],
                                    op=ALU.mult)
            nc.vector.tensor_scalar(out=negmur[:], in0=negmur[:], scalar1=-1.0,
                                    scalar2=None, op0=ALU.mult)
            # norm11 = x*r + (-mu*r)  [(optionally) * w + b]
            n11 = mpool.tile([P, D], F32, tag="n11")
            nc.vector.tensor_scalar(
                out=n11[:], in0=x[:, L - 1, :], scalar1=r11[:], scalar2=negmur[:],
                op0=ALU.mult, op1=ALU.add,
            )
            if use_affine:
                nc.vector.tensor_tensor(out=n11[:], in0=n11[:], in1=lnw_bc[:],
                                        op=ALU.mult)
                nc.vector.tensor_tensor(out=n11[:], in0=n11[:], in1=lnb_bc[:],
                                        op=ALU.add)
            # transpose norm11 -> n11t [d, pos] in 128x128 blocks
            n11t = mpool.tile([P, D], F32, tag="n11t")
            for half in range(2):
                pt = pT.tile([P, 512], F32, tag="pT")
                for cc in range(4):
                    c = half * 4 + cc
                    nc.tensor.transpose(
                        pt[:, cc * P:(cc + 1) * P], n11[:, c * P:(c + 1) * P],
                        ident[:])
                nc.scalar.copy(n11t[:, half * 512:(half + 1) * 512], pt[:])
            # q^T [dk, pos]: two 128-row halves of DK
            pq = pQ.tile([P, 2 * P], F32, tag="pq")
            for h in range(2):
                for c in range(8):
                    nc.tensor.matmul(
                        pq[:, h * P:(h + 1) * P],
                        lhsT=wqt[:, c * DK + h * P: c * DK + (h + 1) * P],
                        rhs=n11t[:, c * P:(c + 1) * P],
                        start=(c == 0), stop=(c == 7),
                    )
            qsb = mpool.tile([P, 2 * P], F32, tag="qsb")
            nc.scalar.copy(qsb[:], pq[:])
            # u [pos, d] = q^T.T @ Wk  (contraction over DK in 2 chunks)
            pu = pU.tile([P, D], F32, tag="pu")
            for h in range(2):
                for nh in range(2):
                    nc.tensor.matmul(
                        pu[:, nh * 512:(nh + 1) * 512],
                        lhsT=qsb[:, h * P:(h + 1) * P],
                        rhs=wk[:, h * D + nh * 512: h * D + (nh + 1) * 512],
                        start=(h == 0), stop=(h == 1),
                    )
            ub = mpool.tile([P, D], BF16, tag="ub")
            if use_affine:
                u32t = mpool.tile([P, D], F32, tag="u32")
                nc.scalar.copy(u32t[:], pu[:])
                # C2 = u . ln_b (before w-scaling)
                c2 = spool.tile([P, 1], F32, tag="c2")
                u16 = mpool.tile([P, D], BF16, tag="u16")
                nc.vector.tensor_copy(u16[:], u32t[:])
                scc2 = scpool.tile([P, D], BF16, tag="sc")
                nc.gpsimd.tensor_tensor(out=scc2[:], in0=u16[:], in1=lnb_bc16[:],
                                        op=ALU.mult)
                nc.vector.tensor_reduce(out=c2[:], in_=scc2[:],
                                        axis=mybir.AxisListType.X, op=ALU.add)
                # uw = u * ln_w
                nc.vector.tensor_tensor(out=u32t[:], in0=u32t[:], in1=lnw_bc[:],
                                        op=ALU.mult)
                nc.vector.tensor_copy(ub[:], u32t[:])
            else:
                nc.scalar.copy(ub[:], pu[:])
            c1 = spool.tile([P, 1], F32, tag="c1")
            nc.vector.tensor_reduce(out=c1[:], in_=ub[:], axis=mybir.AxisListType.X,
                                    op=ALU.add)

            # ---------------- per-layer stats + dots ----------------
            # A[l] = ub . xb_l: product on GpSimd (separate engine), reduce on DVE
            pr11 = scpool.tile([P, D], BF16, tag="sc")
            nc.gpsimd.tensor_tensor(out=pr11[:], in0=xb11[:], in1=ub[:], op=ALU.mult)
            nc.vector.tensor_reduce(out=acol[:, L - 1:L], in_=pr11[:],
                                    axis=mybir.AxisListType.X, op=ALU.add)
            for l in range(L - 1):
                xb = bpool.tile([P, D], BF16, tag="xb")
                nc.scalar.copy(xb[:], x[:, l, :])
                nc.vector.bn_stats(st[:, l, 0:6], xb[:, 0:512])
                nc.vector.bn_stats(st[:, l, 6:12], xb[:, 512:1024])
                nc.vector.bn_aggr(ag[:, l, :], st[:, l, :])
                pr = scpool.tile([P, D], BF16, tag="sc")
                nc.gpsimd.tensor_tensor(out=pr[:], in0=xb[:], in1=ub[:], op=ALU.mult)
                nc.vector.tensor_reduce(out=acol[:, l:l + 1], in_=pr[:],
                                        axis=mybir.AxisListType.X, op=ALU.add)

            # ---------------- rsqrt for all layers ----------------
            mu = ag[:, :, 0]                             # strided [128, L] view
            vpe = spool.tile([P, L], F32, tag="vpe")
            nc.vector.tensor_scalar(out=vpe[:], in0=ag[:, :, 1], scalar1=LN_EPS,
                                    scalar2=None, op0=ALU.add)
            rr = spool.tile([P, L], F32, tag="rr")
            _rsqrt_newton(nc, spool, vpe, rr, L)

            # ---------------- logits + softmax + gate fold ----------------
            lg = spool.tile([P, L], F32, tag="lg")
            nc.vector.tensor_tensor(out=lg[:], in0=acol[:], in1=rr[:], op=ALU.mult)
            mur = spool.tile([P, L], F32, tag="mur")
            nc.vector.tensor_tensor(out=mur[:], in0=mu, in1=rr[:], op=ALU.mult)
            nc.vector.tensor_scalar(out=mur[:], in0=mur[:], scalar1=c1[:],
                                    scalar2=None, op0=ALU.mult)
            nc.vector.tensor_tensor(out=lg[:], in0=lg[:], in1=mur[:],
                                    op=ALU.subtract)
            if use_affine:
                nc.vector.tensor_scalar(out=lg[:], in0=lg[:], scalar1=c2[:],
                                        scalar2=None, op0=ALU.add)
            negmax = spool.tile([P, 1], F32, tag="negmax")
            nc.vector.tensor_reduce(out=negmax[:], in_=lg[:],
                                    axis=mybir.AxisListType.X, op=ALU.max,
                                    negate=True)
            nc.vector.tensor_scalar(out=negmax[:], in0=negmax[:], scalar1=SCALE,
                                    scalar2=None, op0=ALU.mult)
            wts = spool.tile([P, L], F32, tag="wts")
            ssum = spool.tile([P, 1], F32, tag="ssum")
            nc.scalar.activation(
                out=wts[:], in_=lg[:], func=ACTF.Exp, bias=negmax[:], scale=SCALE,
                accum_out=ssum[:],
            )
            rs = spool.tile([P, 1], F32, tag="rs")
            nc.vector.reciprocal(rs[:], ssum[:])
            nc.vector.tensor_scalar(out=rs[:], in0=rs[:], scalar1=(1.0 - g),
                                    scalar2=None, op0=ALU.mult)
            nc.vector.tensor_scalar(out=wts[:], in0=wts[:], scalar1=rs[:],
                                    scalar2=None, op0=ALU.mult)
            nc.vector.tensor_scalar(out=wts[:, L - 1:L], in0=wts[:, L - 1:L],
                                    scalar1=g, scalar2=None, op0=ALU.add)

            # ---------------- mixed: PSUM-accumulated diag matmuls ----------------
            pm = pM.tile([P, D], F32, tag="pm")
            for l in range(L):
                dg = mpool.tile([P, P], F32, tag="dg")
                nc.vector.tensor_scalar(out=dg[:], in0=ident[:],
                                        scalar1=wts[:, l:l + 1], scalar2=None,
                                        op0=ALU.mult)
                for nh in range(2):
                    nc.tensor.matmul(
                        pm[:, nh * 512:(nh + 1) * 512],
                        lhsT=dg[:],
                        rhs=x[:, l, nh * 512:(nh + 1) * 512],
                        start=(l == 0), stop=(l == L - 1),
                    )
            osb = opool.tile([P, D], F32, tag="osb")
            nc.scalar.copy(osb[:], pm[:])
            nc.sync.dma_start(out_dram[r0:r0 + P, :], osb[:])

    nc.compile()
    return nc


_PROGRAM_CACHE = {}


def _get_program(npc, gate, use_affine):
    key = (npc, round(float(gate), 10), bool(use_affine))
    if key not in _PROGRAM_CACHE:
        _PROGRAM_CACHE[key] = build_program(npc, gate, use_affine)
    return _PROGRAM_CACHE[key]


def kernel(states, Wq, Wk, ln_weight, ln_bias, latest_gate, **_unused):
    states = np.ascontiguousarray(np.asarray(states, dtype=np.float32))
    Wq = np.asarray(Wq, dtype=np.float32)
    Wk = np.asarray(Wk, dtype=np.float32)
    ln_weight = np.asarray(ln_weight, dtype=np.float32)
    ln_bias = np.asarray(ln_bias, dtype=np.float32)
    gate = 1.0 / (1.0 + math.exp(-float(np.asarray(latest_gate))))

    use_affine = not (np.all(ln_weight == 1.0) and np.all(ln_bias == 0.0))
    nc = _get_program(NPC, gate, use_affine)

    # host-side prep of the (replicated) small params
    wqt = np.ascontiguousarray(
        Wq.T.reshape(8, P, DK).transpose(1, 0, 2).reshape(P, 8 * DK))
    wkr = np.ascontiguousarray(
        Wk.reshape(2, P, D).transpose(1, 0, 2).reshape(P, 2 * D))

    xs = states.reshape(L, NTOT, D)
    in_maps = []
    for c in range(N_CORES):
        m = {
            "states_shard": np.ascontiguousarray(xs[:, c * NPC:(c + 1) * NPC, :]),
            "wqt": wqt,
            "wk": wkr,
        }
        if use_affine:
            m["lnw"] = ln_weight.reshape(1, D)
            m["lnb"] = ln_bias.reshape(1, D)
        in_maps.append(m)

    res = run_bass_kernel_spmd(nc, in_maps, list(range(N_CORES)))
    out = np.concatenate([res.results[c]["out"] for c in range(N_CORES)], axis=0)
    return np.ascontiguousarray(out.reshape(B, S, D).astype(np.float32))
